# revision 15
# baseline (speedup 1.0000x reference)
"""SMOTE.generate kernel for 8 TRN2 NeuronCores (Bass/Tile).

Problem: X [8192, 512] f32 -> pairwise sq-dists -> per-row 4 nearest
non-self neighbors -> pick by nn_choice -> synth = X + gaps*(X[sel]-X).
Output [32768, 512] f32.

Strategy (data-parallel over rows, 1024 rows/core):
  - s[r, c] = 2*x_r . x_c - |x_c|^2  has the same per-row ordering as
    -dist (per-row constant |x_r|^2 dropped; sqrt monotone).  Self is
    always the row max (|x_r|^2 vs ~ -|x_c|^2), matching the reference's
    top-1-is-self behavior.
  - GEMM on TensorE in fp32r (bf16-pair datapath, 4x faster than fp32) or
    bf16x3 (exact hi/lo split) / fp32 fallbacks; -|x_c|^2 enters as a
    rank-3 bf16 matmul (ones x [hi;lo;lo2] split of -sq).
  - Per 128-row block: DVE max8 + find_index8 over each 4096-col half,
    merge the 16 candidates, one-hot select by nn_choice, indirect-DMA
    gather X[sel], interpolate exactly in fp32.
"""
import os
import sys

import numpy as np

sys.path.insert(0, "/opt/trn_rl_repo")

T, D, N, KNN = 8192, 512, 4, 5
NCORES = 8
R = T // NCORES          # 1024 rows per core
P = 128
RB = R // P              # 8 row blocks per core
HALVES = 2
CH = T // HALVES         # 4096 columns per half
NB = 512                 # matmul free dim (one PSUM bank of fp32)
CB = CH // NB            # 8 col blocks per half
KC = D // P              # 4 contraction chunks of 128
DA = 528                 # gather row: x (512) | -sq (1) | pad; 64B-aligned rows

MODE = os.environ.get("SMOTE_MODE", "v2")  # v2 | bf16x3 | fp32r | fp32r_rr | fp32

_cache = {}


def _build_v2(v2dt="bf16", use_ttr=True, v2sdt="bf16", multigather=False):
    """Single-pass low-precision GEMM shortlist + exact fp32 re-rank.

    s = 2*x_r.x_c - |x_c|^2 computed once in fp16 (1 cyc/row on PE, 3x
    cheaper than bf16x3).  PSUM is cast-copied to fp16 SBUF (+512 shift
    keeps values small for finer quantization).  DVE max8/find_index8
    gives an 8-wide shortlist per row (slot 0 is always self).  The 7
    non-self candidates are gathered in fp32 and re-ranked exactly with
    fused mul+reduce dot products, which restores the reference's fp32
    ordering (host sim: 0/32768 rows differ).
    """
    import concourse.bass as bass
    import concourse.bacc as bacc
    import concourse.mybir as mybir
    import concourse.tile as tile

    dt = mybir.dt
    AF = mybir.ActivationFunctionType
    ALU = mybir.AluOpType
    nc = bacc.Bacc("TRN2", target_bir_lowering=False, debug=False)

    mmdt = dt.float16 if v2dt == "fp16" else dt.bfloat16
    sdt = {"fp16": dt.float16, "bf16": dt.bfloat16, "fp32": dt.float32}[v2sdt]
    sbufs = 1 if v2sdt == "fp32" else 2  # fp32 s is 32KB/partition
    NCAND = int(os.environ.get("SMOTE_V2_NC", "5"))  # non-self shortlist slots

    XTH = nc.dram_tensor("XTH", [D, T], mmdt, kind="ExternalInput").ap()
    XLT2H = nc.dram_tensor("XLT2H", [D, R], mmdt, kind="ExternalInput").ap()
    NEG3 = nc.dram_tensor("NEG3", [3, T], mmdt, kind="ExternalInput").ap()
    ONES3 = nc.dram_tensor("ONES3", [3, P], mmdt, kind="ExternalInput").ap()
    XAUG = nc.dram_tensor("XAUG", [T, DA], dt.float32, kind="ExternalInput").ap()
    XB2A = nc.dram_tensor("XB2A", [R, DA], dt.float32, kind="ExternalInput").ap()
    X = nc.dram_tensor("X", [T, D], dt.float32, kind="ExternalInput").ap()
    XL = nc.dram_tensor("XL", [R, D], dt.float32, kind="ExternalInput").ap()
    GAPS = nc.dram_tensor("GAPS", [R, N], dt.float32, kind="ExternalInput").ap()
    NCHF = nc.dram_tensor("NCHF", [R, N], dt.float32, kind="ExternalInput").ap()
    IOTA8 = nc.dram_tensor("IOTA8", [P, 8], dt.float32, kind="ExternalInput").ap()
    OUT = nc.dram_tensor("OUT", [R * N, D], dt.float32, kind="ExternalOutput").ap()
    OUT3 = OUT.rearrange("(r n) d -> r n d", n=N)

    with tile.TileContext(nc) as tc:
        with (
            tc.tile_pool(name="const", bufs=1) as const,
            tc.tile_pool(name="wk", bufs=2) as wk,
            tc.tile_pool(name="io", bufs=2) as io,
            tc.tile_pool(name="ps", bufs=2, space="PSUM") as ps,
        ):
            # ---- resident operands: X^T fp16 in 4x4 chunks, local 2X^T ----
            CCH = 2048
            NG = T // CCH
            xlt = [const.tile([P, R], mmdt, name=f"xlt{k}") for k in range(KC)]
            xt = [[const.tile([P, CCH], mmdt, name=f"xt{k}_{g}") for g in range(NG)]
                  for k in range(KC)]
            for k in range(KC):
                nc.sync.dma_start(xlt[k][:], XLT2H[k * P:(k + 1) * P, :])
            for k in range(KC):
                nc.sync.dma_start(xt[k][0][:], XTH[k * P:(k + 1) * P, 0:CCH])
            neg3 = const.tile([3, T], mmdt)
            ones3 = const.tile([3, P], mmdt)
            nc.sync.dma_start(neg3[:], NEG3[:])
            nc.sync.dma_start(ones3[:], ONES3[:])
            for g in range(1, NG):
                for k in range(KC):
                    nc.sync.dma_start(xt[k][g][:], XTH[k * P:(k + 1) * P, g * CCH:(g + 1) * CCH])

            iota8 = const.tile([P, 8], dt.float32)
            nc.sync.dma_start(iota8[:], IOTA8[:])

            def stage_a_front(rb):
                """GEMM -> cast."""
                m0 = rb * P
                s16 = wk.tile([P, T], sdt, name=f"s16_{rb}", tag="s16", bufs=sbufs)
                for pg in range(NG):
                    pt = ps.tile([P, CCH], dt.float32, name=f"pt_{rb}_{pg}", tag="pt")
                    for k in range(KC):
                        for cbi in range(CCH // NB):
                            gb = cbi * NB
                            nc.tensor.matmul(pt[:, gb:gb + NB], lhsT=xlt[k][:, m0:m0 + P],
                                             rhs=xt[k][pg][:, gb:gb + NB],
                                             start=(k == 0), stop=False,
                                             skip_group_check=True)
                    for cbi in range(CCH // NB):
                        gb = cbi * NB
                        b0 = pg * CCH + gb
                        nc.tensor.matmul(pt[:, gb:gb + NB], lhsT=ones3[:, :],
                                         rhs=neg3[:, b0:b0 + NB], start=False, stop=True,
                                         skip_group_check=True)
                    nc.scalar.activation(s16[:, pg * CCH:(pg + 1) * CCH], pt[:],
                                         AF.Copy, bias=512.0, scale=1.0)
                return dict(s16=s16, m0=m0)

            def stage_a_back(rb, st):
                """top-8 -> launch candidate gathers -> per-block loads."""
                s16, m0 = st["s16"], st["m0"]
                vals8 = wk.tile([P, 8], sdt, name=f"v8_{rb}", tag="v8")
                idxu = wk.tile([P, 8], dt.uint32, name=f"iu_{rb}", tag="iu")
                nc.vector.max(out=vals8[:], in_=s16[:])
                nc.vector.max_index(out=idxu[:], in_max=vals8[:], in_values=s16[:])

                xg = io.tile([P, NCAND, DA], dt.float32, name=f"xg_{rb}", tag="xg")
                for j in range(NCAND):
                    nc.gpsimd.indirect_dma_start(
                        out=xg[:, j, :], out_offset=None, in_=XAUG[:],
                        in_offset=bass.IndirectOffsetOnAxis(ap=idxu[:, j + 1:j + 2], axis=0))
                gidxf = wk.tile([P, 8], dt.float32, name=f"gx_{rb}", tag="gx")
                nc.gpsimd.tensor_copy(gidxf[:], idxu[:])
                xb2a = io.tile([P, DA], dt.float32, name=f"xb2a_{rb}", tag="xb2a")
                nc.sync.dma_start(xb2a[:], XB2A[m0:m0 + P, :])
                ncf = io.tile([P, N], dt.float32, name=f"ncf_{rb}", tag="ncf")
                nc.sync.dma_start(ncf[:], NCHF[m0:m0 + P, :])
                gaps_t = io.tile([P, N], dt.float32, name=f"gp_{rb}", tag="gp")
                nc.sync.dma_start(gaps_t[:], GAPS[m0:m0 + P, :])
                xb = io.tile([P, D], dt.float32, name=f"xb_{rb}", tag="xb")
                nc.sync.dma_start(xb[:], XL[m0:m0 + P, :])
                # ht_n = (1-g_n)*xb depends only on loads: issue in stage A
                hfac = wk.tile([P, N], dt.float32, name=f"hf_{rb}", tag="hf")
                nc.gpsimd.tensor_scalar(out=hfac[:], in0=gaps_t[:], scalar1=-1.0,
                                        scalar2=1.0, op0=ALU.mult, op1=ALU.add)
                hts = []
                for n in range(N):
                    ht = io.tile([P, D], dt.float32, name=f"ht_{rb}_{n}", tag="ht", bufs=2)
                    nc.scalar.activation(ht[:], xb[:], AF.Copy, scale=hfac[:, n:n + 1])
                    hts.append(ht)
                st.update(idxu=idxu, xg=xg, xb2a=xb2a, ncf=ncf, gaps_t=gaps_t,
                          xb=xb, gidxf=gidxf, hts=hts)
                return st

            def stage_b1(rb, st):
                """Exact re-rank -> rank-compare map -> selected indices."""
                idxu, xg, xb2a = st["idxu"], st["xg"], st["xb2a"]
                ncf, gaps_t, xb, m0 = st["ncf"], st["gaps_t"], st["xb"], st["m0"]
                gidxf = st["gidxf"]
                # batched exact dot products: one wide mul, per-candidate ACT reduce
                scrB = wk.tile([P, NCAND, DA], dt.float32, name=f"scrB_{rb}", tag="scrB")
                nc.vector.tensor_mul(scrB[:, :, :], xg[:, :, :],
                                     xb2a[:, None, :].broadcast_to([P, NCAND, DA]))
                sex = wk.tile([P, 8], dt.float32, name=f"sex_{rb}", tag="sex")
                for j in range(NCAND):
                    scr2 = wk.tile([P, DA], dt.float32, name=f"scr2_{rb}_{j}", tag="scr2")
                    nc.scalar.activation(scr2[:], scrB[:, j, :], AF.Copy,
                                         accum_out=sex[:, j:j + 1])

                # rank each candidate by pairwise compares (no sort needed):
                # rank[j] = #{j': sex[j'] > sex[j]};  sel[r,n] = gidx[1+j] where
                # rank[j] == nnc[r,n]
                q3 = wk.tile([P, NCAND, NCAND], dt.float32, name=f"q3_{rb}", tag="q3")
                nc.vector.tensor_tensor(q3[:, :, :],
                                        sex[:, None, :NCAND].broadcast_to([P, NCAND, NCAND]),
                                        sex[:, :NCAND, None].broadcast_to([P, NCAND, NCAND]),
                                        ALU.is_gt)
                rank = wk.tile([P, NCAND], dt.float32, name=f"rk_{rb}", tag="rk")
                nc.vector.tensor_reduce(out=rank[:, :], in_=q3[:, :, :],
                                        axis=mybir.AxisListType.X, op=ALU.add)
                q4 = wk.tile([P, N, NCAND], dt.float32, name=f"q4_{rb}", tag="q4")
                nc.vector.tensor_tensor(q4[:, :, :],
                                        rank[:, None, :].broadcast_to([P, N, NCAND]),
                                        ncf[:, :, None].broadcast_to([P, N, NCAND]),
                                        ALU.is_equal)
                nc.vector.tensor_mul(q4[:, :, :], q4[:, :, :],
                                     gidxf[:, None, 1:1 + NCAND].broadcast_to([P, N, NCAND]))
                self_f = wk.tile([P, N], dt.float32, name=f"sf_{rb}", tag="sf")
                nc.vector.tensor_reduce(out=self_f[:, :], in_=q4[:, :, :],
                                        axis=mybir.AxisListType.X, op=ALU.add)
                selu = wk.tile([P, N], dt.uint32, name=f"su_{rb}", tag="su")
                nc.gpsimd.tensor_copy(selu[:], self_f[:])
                st["selu"] = selu

            def stage_b2(rb, st):
                """Gather selected rows, interpolate, store."""
                selu, gaps_t, m0, hts = st["selu"], st["gaps_t"], st["m0"], st["hts"]
                xs4 = io.tile([P, N, D], dt.float32, name=f"xs4_{rb}", tag="xs4")
                for n in range(N):
                    nc.gpsimd.indirect_dma_start(
                        out=xs4[:, n, :], out_offset=None, in_=X[:],
                        in_offset=bass.IndirectOffsetOnAxis(ap=selu[:, n:n + 1], axis=0))
                    df = io.tile([P, D], dt.float32, name=f"df_{rb}_{n}", tag="df", bufs=2)
                    nc.scalar.activation(df[:], xs4[:, n, :], AF.Copy,
                                         scale=gaps_t[:, n:n + 1])
                    ot = io.tile([P, D], dt.float32, name=f"ot_{rb}_{n}", tag="ot", bufs=2)
                    nc.gpsimd.tensor_add(ot[:], df[:], hts[n][:])
                    nc.sync.dma_start(OUT3[m0:m0 + P, n, :], ot[:])

            # software pipeline: per iteration emit GEMM(rb), then the previous
            # block's re-rank (so its V-ops precede maxes(rb) in the queue),
            # then its interp, then maxes+gathers(rb)
            prev = stage_a_back(0, stage_a_front(0))
            for rb in range(1, RB):
                st = stage_a_front(rb)
                stage_b1(rb - 1, prev)
                stage_b2(rb - 1, prev)
                prev = stage_a_back(rb, st)
            stage_b1(RB - 1, prev)
            stage_b2(RB - 1, prev)

    nc.compile()
    return nc


def _bf16(x):
    import ml_dtypes
    return x.astype(ml_dtypes.bfloat16)


def _pair_round(x):
    hi = _bf16(x).astype(np.float32)
    lo = _bf16(x - hi).astype(np.float32)
    return hi + lo


V2DT = os.environ.get("SMOTE_V2_DT", "fp16")
V2TTR = os.environ.get("SMOTE_V2_TTR", "0") == "1"
V2SDT = os.environ.get("SMOTE_V2_SDT", "fp16")
V2MG = os.environ.get("SMOTE_V2_MG", "0") == "1"


def _get_nc(mode):
    key = (mode, V2DT, V2TTR, V2SDT, V2MG, os.environ.get("SMOTE_V2_NC", "5")) if mode == "v2" else mode
    if key not in _cache:
        _cache[key] = _build_v2(V2DT, V2TTR, V2SDT, V2MG) if mode == "v2" else _build(mode)
    return _cache[key]


def _kernel_v2(X, gaps, nnc):
    from concourse.bass_utils import run_bass_kernel_spmd

    nc = _get_nc("v2")

    sq = np.einsum("td,td->t", X, X, dtype=np.float32).astype(np.float32)
    negsq = -sq
    if V2DT == "fp16":
        f16 = lambda a: a.astype(np.float16)
    else:
        import ml_dtypes
        f16 = lambda a: a.astype(ml_dtypes.bfloat16)
    n1 = f16(negsq).astype(np.float32)
    n2 = f16(negsq - n1).astype(np.float32)
    n3 = f16(negsq - n1 - n2).astype(np.float32)
    NEG3 = np.ascontiguousarray(np.stack([f16(n1), f16(n2), f16(n3)]))
    ONES3 = np.ascontiguousarray(f16(np.ones((3, P), dtype=np.float32)))
    XTH = np.ascontiguousarray(f16(X.T))
    xaug = np.zeros((T, DA), dtype=np.float32)
    xaug[:, :D] = X
    xaug[:, D] = negsq
    iota8 = np.broadcast_to(np.arange(8, dtype=np.float32)[None, :], (P, 8)).copy()
    common = dict(XTH=XTH, NEG3=NEG3, ONES3=ONES3, XAUG=xaug, X=X, IOTA8=iota8)

    in_maps = []
    for c in range(NCORES):
        r0 = c * R
        xl = X[r0:r0 + R]
        m = dict(common)
        m["XLT2H"] = np.ascontiguousarray(f16((2.0 * xl).T))
        m["XL"] = np.ascontiguousarray(xl)
        xb2a = np.zeros((R, DA), dtype=np.float32)
        xb2a[:, :D] = 2.0 * xl
        xb2a[:, D] = 1.0
        m["XB2A"] = xb2a
        m["GAPS"] = np.ascontiguousarray(gaps[r0:r0 + R])
        m["NCHF"] = np.ascontiguousarray(nnc[r0:r0 + R].astype(np.float32))
        in_maps.append(m)
    return nc, in_maps


def kernel(X, gaps, nn_choice, k, _want_results=False, _trace=False):
    X = np.ascontiguousarray(np.asarray(X, dtype=np.float32))
    gaps = np.ascontiguousarray(np.asarray(gaps, dtype=np.float32))
    nnc = np.asarray(nn_choice).astype(np.int64)
    assert int(k) == KNN and X.shape == (T, D) and gaps.shape == (T, N)

    from concourse.bass_utils import run_bass_kernel_spmd

    mode = MODE
    if mode == "v2":
        nc, in_maps = _kernel_v2(X, gaps, nnc)
        res = run_bass_kernel_spmd(nc, in_maps, core_ids=list(range(NCORES)), trace=_trace)
        out = np.concatenate([res.results[c]["OUT"] for c in range(NCORES)], axis=0)
        if _want_results:
            return out, res
        return out
    nc = _get_nc(mode)

    sq = np.einsum("td,td->t", X, X, dtype=np.float32).astype(np.float32)
    negsq = -sq
    n1 = _bf16(negsq).astype(np.float32)
    n2 = _bf16(negsq - n1).astype(np.float32)
    n3 = _bf16(negsq - n1 - n2).astype(np.float32)
    NEG3 = np.ascontiguousarray(np.stack([_bf16(n1), _bf16(n2), _bf16(n3)]))
    ONES3 = np.ascontiguousarray(np.ones((3, P), dtype=np.float32).astype(NEG3.dtype))
    XTc = np.ascontiguousarray(X.T)

    common = dict(NEG3=NEG3, ONES3=ONES3, X=X)
    if mode == "fp32r_rr":
        xaug = np.zeros((T, DA), dtype=np.float32)
        xaug[:, :D] = X
        xaug[:, D] = negsq
        common["XAUG"] = xaug
    if mode in ("fp32r", "fp32r_rr"):
        common["XT"] = np.ascontiguousarray(_pair_round(XTc))
    elif mode == "fp32":
        common["XT"] = XTc
    else:
        xth = _bf16(XTc)
        common["XTH"] = np.ascontiguousarray(xth)
        common["XTL"] = np.ascontiguousarray(_bf16(XTc - xth.astype(np.float32)))

    in_maps = []
    for c in range(NCORES):
        r0 = c * R
        xl = X[r0:r0 + R]
        xlt2 = np.ascontiguousarray((2.0 * xl).T)
        m = dict(common)
        if mode in ("fp32r", "fp32r_rr"):
            m["XLT2"] = np.ascontiguousarray(_pair_round(xlt2))
        elif mode == "fp32":
            m["XLT2"] = xlt2
        else:
            h = _bf16(xlt2)
            m["XLT2H"] = np.ascontiguousarray(h)
            m["XLT2L"] = np.ascontiguousarray(_bf16(xlt2 - h.astype(np.float32)))
        m["XL"] = np.ascontiguousarray(xl)
        if mode == "fp32r_rr":
            xb2a = np.zeros((R, DA), dtype=np.float32)
            xb2a[:, :D] = 2.0 * xl
            xb2a[:, D] = 1.0
            m["XB2A"] = xb2a
        m["GAPS"] = np.ascontiguousarray(gaps[r0:r0 + R])
        m["NCHF"] = np.ascontiguousarray(nnc[r0:r0 + R].astype(np.float32))
        in_maps.append(m)

    res = run_bass_kernel_spmd(nc, in_maps, core_ids=list(range(NCORES)), trace=_trace)
    out = np.concatenate([res.results[c]["OUT"] for c in range(NCORES)], axis=0)
    if _want_results:
        return out, res
    return out



# revision 17
# speedup vs baseline: 1.0215x; 1.0215x over previous
"""SMOTE.generate kernel for 8 TRN2 NeuronCores (Bass/Tile).

Problem: X [8192, 512] f32 -> pairwise sq-dists -> per-row 4 nearest
non-self neighbors -> pick by nn_choice -> synth = X + gaps*(X[sel]-X).
Output [32768, 512] f32.

Strategy (data-parallel over rows, 1024 rows/core):
  - s[r, c] = 2*x_r . x_c - |x_c|^2  has the same per-row ordering as
    -dist (per-row constant |x_r|^2 dropped; sqrt monotone).  Self is
    always the row max (|x_r|^2 vs ~ -|x_c|^2), matching the reference's
    top-1-is-self behavior.
  - GEMM on TensorE in fp32r (bf16-pair datapath, 4x faster than fp32) or
    bf16x3 (exact hi/lo split) / fp32 fallbacks; -|x_c|^2 enters as a
    rank-3 bf16 matmul (ones x [hi;lo;lo2] split of -sq).
  - Per 128-row block: DVE max8 + find_index8 over each 4096-col half,
    merge the 16 candidates, one-hot select by nn_choice, indirect-DMA
    gather X[sel], interpolate exactly in fp32.
"""
import os
import sys

import numpy as np

sys.path.insert(0, "/opt/trn_rl_repo")

T, D, N, KNN = 8192, 512, 4, 5
NCORES = 8
R = T // NCORES          # 1024 rows per core
P = 128
RB = R // P              # 8 row blocks per core
HALVES = 2
CH = T // HALVES         # 4096 columns per half
NB = 512                 # matmul free dim (one PSUM bank of fp32)
CB = CH // NB            # 8 col blocks per half
KC = D // P              # 4 contraction chunks of 128
DA = 528                 # gather row: x (512) | -sq (1) | pad; 64B-aligned rows

MODE = os.environ.get("SMOTE_MODE", "v2")  # v2 | bf16x3 | fp32r | fp32r_rr | fp32

_cache = {}


def _build_v2(v2dt="bf16", use_ttr=True, v2sdt="bf16", multigather=False):
    """Single-pass low-precision GEMM shortlist + exact fp32 re-rank.

    s = 2*x_r.x_c - |x_c|^2 computed once in fp16 (1 cyc/row on PE, 3x
    cheaper than bf16x3).  PSUM is cast-copied to fp16 SBUF (+512 shift
    keeps values small for finer quantization).  DVE max8/find_index8
    gives an 8-wide shortlist per row (slot 0 is always self).  The 7
    non-self candidates are gathered in fp32 and re-ranked exactly with
    fused mul+reduce dot products, which restores the reference's fp32
    ordering (host sim: 0/32768 rows differ).
    """
    import concourse.bass as bass
    import concourse.bacc as bacc
    import concourse.mybir as mybir
    import concourse.tile as tile

    dt = mybir.dt
    AF = mybir.ActivationFunctionType
    ALU = mybir.AluOpType
    nc = bacc.Bacc("TRN2", target_bir_lowering=False, debug=False)

    mmdt = dt.float16 if v2dt == "fp16" else dt.bfloat16
    sdt = {"fp16": dt.float16, "bf16": dt.bfloat16, "fp32": dt.float32}[v2sdt]
    sbufs = 1 if v2sdt == "fp32" else 2  # fp32 s is 32KB/partition
    NCAND = int(os.environ.get("SMOTE_V2_NC", "5"))  # non-self shortlist slots

    XTH = nc.dram_tensor("XTH", [D, T], mmdt, kind="ExternalInput").ap()
    XLT2H = nc.dram_tensor("XLT2H", [D, R], mmdt, kind="ExternalInput").ap()
    NEG3 = nc.dram_tensor("NEG3", [3, T], mmdt, kind="ExternalInput").ap()
    ONES3 = nc.dram_tensor("ONES3", [3, P], mmdt, kind="ExternalInput").ap()
    XAUG = nc.dram_tensor("XAUG", [T, DA], dt.float32, kind="ExternalInput").ap()
    XB2A = nc.dram_tensor("XB2A", [R, DA], dt.float32, kind="ExternalInput").ap()
    X = nc.dram_tensor("X", [T, D], dt.float32, kind="ExternalInput").ap()
    XL = nc.dram_tensor("XL", [R, D], dt.float32, kind="ExternalInput").ap()
    GAPS = nc.dram_tensor("GAPS", [R, N], dt.float32, kind="ExternalInput").ap()
    NCHF = nc.dram_tensor("NCHF", [R, N], dt.float32, kind="ExternalInput").ap()
    IOTA8 = nc.dram_tensor("IOTA8", [P, 8], dt.float32, kind="ExternalInput").ap()
    OUT = nc.dram_tensor("OUT", [R * N, D], dt.float32, kind="ExternalOutput").ap()
    OUT3 = OUT.rearrange("(r n) d -> r n d", n=N)

    with tile.TileContext(nc) as tc:
        with (
            tc.tile_pool(name="const", bufs=1) as const,
            tc.tile_pool(name="wk", bufs=2) as wk,
            tc.tile_pool(name="io", bufs=2) as io,
            tc.tile_pool(name="ps", bufs=2, space="PSUM") as ps,
        ):
            # ---- resident operands: X^T fp16 in 4x4 chunks, local 2X^T ----
            CCH = 2048
            NG = T // CCH
            xlt = [const.tile([P, R], mmdt, name=f"xlt{k}") for k in range(KC)]
            xt = [[const.tile([P, CCH], mmdt, name=f"xt{k}_{g}") for g in range(NG)]
                  for k in range(KC)]
            for k in range(KC):
                nc.sync.dma_start(xlt[k][:], XLT2H[k * P:(k + 1) * P, :])
            for k in range(KC):
                nc.sync.dma_start(xt[k][0][:], XTH[k * P:(k + 1) * P, 0:CCH])
            neg3 = const.tile([3, T], mmdt)
            ones3 = const.tile([3, P], mmdt)
            nc.sync.dma_start(neg3[:], NEG3[:])
            nc.sync.dma_start(ones3[:], ONES3[:])
            for g in range(1, NG):
                for k in range(KC):
                    nc.sync.dma_start(xt[k][g][:], XTH[k * P:(k + 1) * P, g * CCH:(g + 1) * CCH])

            iota8 = const.tile([P, 8], dt.float32)
            nc.sync.dma_start(iota8[:], IOTA8[:])

            def stage_a_front(rb):
                """GEMM -> cast."""
                m0 = rb * P
                s16 = wk.tile([P, T], sdt, name=f"s16_{rb}", tag="s16", bufs=sbufs)
                for pg in range(NG):
                    pt = ps.tile([P, CCH], dt.float32, name=f"pt_{rb}_{pg}", tag="pt")
                    for k in range(KC):
                        for cbi in range(CCH // NB):
                            gb = cbi * NB
                            nc.tensor.matmul(pt[:, gb:gb + NB], lhsT=xlt[k][:, m0:m0 + P],
                                             rhs=xt[k][pg][:, gb:gb + NB],
                                             start=(k == 0), stop=False,
                                             skip_group_check=True)
                    for cbi in range(CCH // NB):
                        gb = cbi * NB
                        b0 = pg * CCH + gb
                        nc.tensor.matmul(pt[:, gb:gb + NB], lhsT=ones3[:, :],
                                         rhs=neg3[:, b0:b0 + NB], start=False, stop=True,
                                         skip_group_check=True)
                    nc.scalar.activation(s16[:, pg * CCH:(pg + 1) * CCH], pt[:],
                                         AF.Copy, bias=512.0, scale=1.0)
                return dict(s16=s16, m0=m0)

            def stage_a_back(rb, st):
                """top-8 -> launch candidate gathers -> per-block loads."""
                s16, m0 = st["s16"], st["m0"]
                vals8 = wk.tile([P, 8], sdt, name=f"v8_{rb}", tag="v8")
                idxu = wk.tile([P, 8], dt.uint32, name=f"iu_{rb}", tag="iu")
                nc.vector.max(out=vals8[:], in_=s16[:])
                nc.vector.max_index(out=idxu[:], in_max=vals8[:], in_values=s16[:])

                xg = io.tile([P, NCAND, DA], dt.float32, name=f"xg_{rb}", tag="xg")
                for j in range(NCAND):
                    nc.gpsimd.indirect_dma_start(
                        out=xg[:, j, :], out_offset=None, in_=XAUG[:],
                        in_offset=bass.IndirectOffsetOnAxis(ap=idxu[:, j + 1:j + 2], axis=0))
                gidxf = wk.tile([P, 8], dt.float32, name=f"gx_{rb}", tag="gx")
                nc.gpsimd.tensor_copy(gidxf[:], idxu[:])
                xb2a = io.tile([P, DA], dt.float32, name=f"xb2a_{rb}", tag="xb2a")
                nc.sync.dma_start(xb2a[:], XB2A[m0:m0 + P, :])
                ncf = io.tile([P, N], dt.float32, name=f"ncf_{rb}", tag="ncf")
                nc.sync.dma_start(ncf[:], NCHF[m0:m0 + P, :])
                gaps_t = io.tile([P, N], dt.float32, name=f"gp_{rb}", tag="gp")
                nc.sync.dma_start(gaps_t[:], GAPS[m0:m0 + P, :])
                xb = io.tile([P, D], dt.float32, name=f"xb_{rb}", tag="xb")
                nc.sync.dma_start(xb[:], XL[m0:m0 + P, :])
                # ht_n = (1-g_n)*xb depends only on loads: issue in stage A
                hfac = wk.tile([P, N], dt.float32, name=f"hf_{rb}", tag="hf")
                nc.gpsimd.tensor_scalar(out=hfac[:], in0=gaps_t[:], scalar1=-1.0,
                                        scalar2=1.0, op0=ALU.mult, op1=ALU.add)
                hts = []
                for n in range(N):
                    ht = io.tile([P, D], dt.float32, name=f"ht_{rb}_{n}", tag="ht", bufs=2)
                    nc.scalar.activation(ht[:], xb[:], AF.Copy, scale=hfac[:, n:n + 1])
                    hts.append(ht)
                st.update(idxu=idxu, xg=xg, xb2a=xb2a, ncf=ncf, gaps_t=gaps_t,
                          xb=xb, gidxf=gidxf, hts=hts)
                return st

            def stage_b1(rb, st):
                """Exact re-rank -> rank-compare map -> selected indices."""
                idxu, xg, xb2a = st["idxu"], st["xg"], st["xb2a"]
                ncf, gaps_t, xb, m0 = st["ncf"], st["gaps_t"], st["xb"], st["m0"]
                gidxf = st["gidxf"]
                # batched exact dot products: one wide mul, per-candidate ACT reduce
                scrB = wk.tile([P, NCAND, DA], dt.float32, name=f"scrB_{rb}", tag="scrB")
                nc.vector.tensor_mul(scrB[:, :, :], xg[:, :, :],
                                     xb2a[:, None, :].broadcast_to([P, NCAND, DA]))
                sex = wk.tile([P, 8], dt.float32, name=f"sex_{rb}", tag="sex")
                for j in range(NCAND):
                    scr2 = wk.tile([P, DA], dt.float32, name=f"scr2_{rb}_{j}", tag="scr2")
                    nc.scalar.activation(scr2[:], scrB[:, j, :], AF.Copy,
                                         accum_out=sex[:, j:j + 1])

                # rank each candidate by pairwise compares (no sort needed):
                # rank[j] = #{j': sex[j'] > sex[j]};  sel[r,n] = gidx[1+j] where
                # rank[j] == nnc[r,n]
                q3 = wk.tile([P, NCAND, NCAND], dt.float32, name=f"q3_{rb}", tag="q3")
                nc.vector.tensor_tensor(q3[:, :, :],
                                        sex[:, None, :NCAND].broadcast_to([P, NCAND, NCAND]),
                                        sex[:, :NCAND, None].broadcast_to([P, NCAND, NCAND]),
                                        ALU.is_gt)
                rank = wk.tile([P, NCAND], dt.float32, name=f"rk_{rb}", tag="rk")
                nc.vector.tensor_reduce(out=rank[:, :], in_=q3[:, :, :],
                                        axis=mybir.AxisListType.X, op=ALU.add)
                q4 = wk.tile([P, N, NCAND], dt.float32, name=f"q4_{rb}", tag="q4")
                nc.vector.tensor_tensor(q4[:, :, :],
                                        rank[:, None, :].broadcast_to([P, N, NCAND]),
                                        ncf[:, :, None].broadcast_to([P, N, NCAND]),
                                        ALU.is_equal)
                nc.vector.tensor_mul(q4[:, :, :], q4[:, :, :],
                                     gidxf[:, None, 1:1 + NCAND].broadcast_to([P, N, NCAND]))
                self_f = wk.tile([P, N], dt.float32, name=f"sf_{rb}", tag="sf")
                nc.vector.tensor_reduce(out=self_f[:, :], in_=q4[:, :, :],
                                        axis=mybir.AxisListType.X, op=ALU.add)
                selu = wk.tile([P, N], dt.uint32, name=f"su_{rb}", tag="su")
                nc.gpsimd.tensor_copy(selu[:], self_f[:])
                st["selu"] = selu

            def stage_b2(rb, st):
                """Gather selected rows, interpolate, store."""
                selu, gaps_t, m0, hts = st["selu"], st["gaps_t"], st["m0"], st["hts"]
                xs4 = io.tile([P, N, D], dt.float32, name=f"xs4_{rb}", tag="xs4")
                for n in range(N):
                    nc.gpsimd.indirect_dma_start(
                        out=xs4[:, n, :], out_offset=None, in_=X[:],
                        in_offset=bass.IndirectOffsetOnAxis(ap=selu[:, n:n + 1], axis=0))
                    df = io.tile([P, D], dt.float32, name=f"df_{rb}_{n}", tag="df", bufs=2)
                    nc.scalar.activation(df[:], xs4[:, n, :], AF.Copy,
                                         scale=gaps_t[:, n:n + 1])
                    ot = io.tile([P, D], dt.float32, name=f"ot_{rb}_{n}", tag="ot", bufs=2)
                    nc.gpsimd.tensor_add(ot[:], df[:], hts[n][:])
                    nc.sync.dma_start(OUT3[m0:m0 + P, n, :], ot[:])

            # software pipeline: emit the full front of block rb (GEMM, casts,
            # maxes, gathers), then the previous block's re-rank + interp
            prev = stage_a_back(0, stage_a_front(0))
            for rb in range(1, RB):
                cur = stage_a_back(rb, stage_a_front(rb))
                stage_b1(rb - 1, prev)
                stage_b2(rb - 1, prev)
                prev = cur
            stage_b1(RB - 1, prev)
            stage_b2(RB - 1, prev)

    nc.compile()
    return nc


def _bf16(x):
    import ml_dtypes
    return x.astype(ml_dtypes.bfloat16)


def _pair_round(x):
    hi = _bf16(x).astype(np.float32)
    lo = _bf16(x - hi).astype(np.float32)
    return hi + lo


V2DT = os.environ.get("SMOTE_V2_DT", "fp16")
V2TTR = os.environ.get("SMOTE_V2_TTR", "0") == "1"
V2SDT = os.environ.get("SMOTE_V2_SDT", "fp16")
V2MG = os.environ.get("SMOTE_V2_MG", "0") == "1"


def _get_nc(mode):
    key = (mode, V2DT, V2TTR, V2SDT, V2MG, os.environ.get("SMOTE_V2_NC", "5")) if mode == "v2" else mode
    if key not in _cache:
        _cache[key] = _build_v2(V2DT, V2TTR, V2SDT, V2MG) if mode == "v2" else _build(mode)
    return _cache[key]


def _kernel_v2(X, gaps, nnc):
    from concourse.bass_utils import run_bass_kernel_spmd

    nc = _get_nc("v2")

    sq = np.einsum("td,td->t", X, X, dtype=np.float32).astype(np.float32)
    negsq = -sq
    if V2DT == "fp16":
        f16 = lambda a: a.astype(np.float16)
    else:
        import ml_dtypes
        f16 = lambda a: a.astype(ml_dtypes.bfloat16)
    n1 = f16(negsq).astype(np.float32)
    n2 = f16(negsq - n1).astype(np.float32)
    n3 = f16(negsq - n1 - n2).astype(np.float32)
    NEG3 = np.ascontiguousarray(np.stack([f16(n1), f16(n2), f16(n3)]))
    ONES3 = np.ascontiguousarray(f16(np.ones((3, P), dtype=np.float32)))
    XTH = np.ascontiguousarray(f16(X.T))
    xaug = np.zeros((T, DA), dtype=np.float32)
    xaug[:, :D] = X
    xaug[:, D] = negsq
    iota8 = np.broadcast_to(np.arange(8, dtype=np.float32)[None, :], (P, 8)).copy()
    common = dict(XTH=XTH, NEG3=NEG3, ONES3=ONES3, XAUG=xaug, X=X, IOTA8=iota8)

    in_maps = []
    for c in range(NCORES):
        r0 = c * R
        xl = X[r0:r0 + R]
        m = dict(common)
        m["XLT2H"] = np.ascontiguousarray(f16((2.0 * xl).T))
        m["XL"] = np.ascontiguousarray(xl)
        xb2a = np.zeros((R, DA), dtype=np.float32)
        xb2a[:, :D] = 2.0 * xl
        xb2a[:, D] = 1.0
        m["XB2A"] = xb2a
        m["GAPS"] = np.ascontiguousarray(gaps[r0:r0 + R])
        m["NCHF"] = np.ascontiguousarray(nnc[r0:r0 + R].astype(np.float32))
        in_maps.append(m)
    return nc, in_maps


def kernel(X, gaps, nn_choice, k, _want_results=False, _trace=False):
    X = np.ascontiguousarray(np.asarray(X, dtype=np.float32))
    gaps = np.ascontiguousarray(np.asarray(gaps, dtype=np.float32))
    nnc = np.asarray(nn_choice).astype(np.int64)
    assert int(k) == KNN and X.shape == (T, D) and gaps.shape == (T, N)

    from concourse.bass_utils import run_bass_kernel_spmd

    mode = MODE
    if mode == "v2":
        nc, in_maps = _kernel_v2(X, gaps, nnc)
        res = run_bass_kernel_spmd(nc, in_maps, core_ids=list(range(NCORES)), trace=_trace)
        out = np.concatenate([res.results[c]["OUT"] for c in range(NCORES)], axis=0)
        if _want_results:
            return out, res
        return out
    nc = _get_nc(mode)

    sq = np.einsum("td,td->t", X, X, dtype=np.float32).astype(np.float32)
    negsq = -sq
    n1 = _bf16(negsq).astype(np.float32)
    n2 = _bf16(negsq - n1).astype(np.float32)
    n3 = _bf16(negsq - n1 - n2).astype(np.float32)
    NEG3 = np.ascontiguousarray(np.stack([_bf16(n1), _bf16(n2), _bf16(n3)]))
    ONES3 = np.ascontiguousarray(np.ones((3, P), dtype=np.float32).astype(NEG3.dtype))
    XTc = np.ascontiguousarray(X.T)

    common = dict(NEG3=NEG3, ONES3=ONES3, X=X)
    if mode == "fp32r_rr":
        xaug = np.zeros((T, DA), dtype=np.float32)
        xaug[:, :D] = X
        xaug[:, D] = negsq
        common["XAUG"] = xaug
    if mode in ("fp32r", "fp32r_rr"):
        common["XT"] = np.ascontiguousarray(_pair_round(XTc))
    elif mode == "fp32":
        common["XT"] = XTc
    else:
        xth = _bf16(XTc)
        common["XTH"] = np.ascontiguousarray(xth)
        common["XTL"] = np.ascontiguousarray(_bf16(XTc - xth.astype(np.float32)))

    in_maps = []
    for c in range(NCORES):
        r0 = c * R
        xl = X[r0:r0 + R]
        xlt2 = np.ascontiguousarray((2.0 * xl).T)
        m = dict(common)
        if mode in ("fp32r", "fp32r_rr"):
            m["XLT2"] = np.ascontiguousarray(_pair_round(xlt2))
        elif mode == "fp32":
            m["XLT2"] = xlt2
        else:
            h = _bf16(xlt2)
            m["XLT2H"] = np.ascontiguousarray(h)
            m["XLT2L"] = np.ascontiguousarray(_bf16(xlt2 - h.astype(np.float32)))
        m["XL"] = np.ascontiguousarray(xl)
        if mode == "fp32r_rr":
            xb2a = np.zeros((R, DA), dtype=np.float32)
            xb2a[:, :D] = 2.0 * xl
            xb2a[:, D] = 1.0
            m["XB2A"] = xb2a
        m["GAPS"] = np.ascontiguousarray(gaps[r0:r0 + R])
        m["NCHF"] = np.ascontiguousarray(nnc[r0:r0 + R].astype(np.float32))
        in_maps.append(m)

    res = run_bass_kernel_spmd(nc, in_maps, core_ids=list(range(NCORES)), trace=_trace)
    out = np.concatenate([res.results[c]["OUT"] for c in range(NCORES)], axis=0)
    if _want_results:
        return out, res
    return out

